# revision 26
# baseline (speedup 1.0000x reference)
"""ConnectivityLoss Trainium2 Bass kernel.

Problem (hardcoded): pred/target (32, 1, 512, 512) f32.
  5 iterations of soft-skeletonize (3x3 min-pool -> 3x3 max-pool ->
  x = x - (M - m); both reference relus are provably no-ops), then 3x3
  sum-pool, endpoint/crossing masks, and a weighted MSE of the three
  pairs.

Sharding: pure data parallel over the batch dim; core i processes image
pairs 4i..4i+3 and returns per-partition partial sums of squared diffs;
the host sums and normalizes.

Per-core layout: partition p (128) owns image rows 4p..4p+3.
Free dims: (side 2, rowslot 4, col 512), fully contiguous.

Everything on-device runs in bf16 (numpy-validated loss error ~5.7e-3
vs the 2e-2 gate).  bf16 keeps every DVE tensor_tensor in the 2x perf
mode (f32 tt runs 1x).  MSE squares+row-sums run on the Scalar engine
(ACT Square with accum_out).

The four chunks are processed as two interleaved streams (chunk pairs
(0,1) and (2,3)): per iteration the emission alternates stream A /
stream B so each stream's TensorE shift-matmul + ACT evacuation
latencies hide under the other stream's DVE block, and each stream's
post-pool overlaps the other's morphology.  Each stream owns its
x/m/M/sk/t5/stage buffers; the hpool pair scratch (tmin/tmax) is
shared (adjacent-op WAR, zero cost).  Post-pool tensors alias the
stream's dead morphology buffers.

Cross-partition row shifts (rows 4p-1 / 4p+4) run on the idle
TensorEngine as shifted-identity bf16 matmuls into PSUM; ScalarE
evacuates PSUM to bf16 SBUF rows via an Identity activation whose
per-partition bias plants +/-BIG sentinels at the image boundary rows
(the shift matrices write exact 0.0 there), so pool combines cover all
128 partitions with no boundary special case.  The hpool combine is
split into slot3 / slot0 / slots1:2 ops so the following vpool's shift
matmuls start ~2us early.
"""
import numpy as np
import ml_dtypes

import concourse.bass as bass
import concourse.tile as tile
from concourse import mybir
from concourse.bass_utils import run_bass_kernel_spmd

F32 = mybir.dt.float32
BF16 = mybir.dt.bfloat16
OP = mybir.AluOpType
AF = mybir.ActivationFunctionType

BIG = 1.0e30
P = 128
NCORES = 8
CHUNKS = 4
H = W = 512
ITERS = 5

_cache = {}


def _split_waits(nc, limit=1):
    """This walrus build rejects instructions with more than ~1 embedded
    sync wait; hoist waits into standalone EventSemaphore instructions."""
    counter = 0
    for fn in nc.m.functions:
        for bb in fn.blocks:
            lst = list(bb.instructions)
            out = []
            changed = False
            for ins in lst:
                si = ins.sync_info
                waits = list(si.on_wait) if si is not None else []
                if len(waits) > limit:
                    changed = True
                    for w in waits:
                        counter += 1
                        es = mybir.InstEventSemaphore(
                            name=f"I-wsplit-{counter}", ins=[], outs=[],
                            sync_info=mybir.SyncInfo(on_wait=[w], on_update=[]),
                            bass_nofuse=True,
                        )
                        es.engine = ins.engine
                        out.append(es)
                    ins.sync_info = mybir.SyncInfo(
                        on_wait=[], on_update=list(si.on_update))
                out.append(ins)
            if changed:
                bb.instructions = out
    return counter


def _shift_mats():
    sup = np.zeros((P, P), np.float32)   # psum[p] = rhs[p-1]; col 0 zero
    sdn = np.zeros((P, P), np.float32)   # psum[p] = rhs[p+1]; col 127 zero
    for p in range(1, P):
        sup[p - 1, p] = 1.0
    for p in range(P - 1):
        sdn[p + 1, p] = 1.0
    bvec = np.zeros((P, 4), np.float32)
    bvec[0, 0] = BIG      # min, shift-up sentinel at row 0
    bvec[127, 1] = BIG    # min, shift-down sentinel at row 511
    bvec[0, 2] = -BIG     # max
    bvec[127, 3] = -BIG
    return (sup.astype(ml_dtypes.bfloat16), sdn.astype(ml_dtypes.bfloat16),
            bvec)


def _build():
    nc = bass.Bass()
    pred = nc.dram_tensor("pred", [CHUNKS, H, W], F32, kind="ExternalInput")
    targ = nc.dram_tensor("targ", [CHUNKS, H, W], F32, kind="ExternalInput")
    supd = nc.dram_tensor("sup", [P, P], BF16, kind="ExternalInput")
    sdnd = nc.dram_tensor("sdn", [P, P], BF16, kind="ExternalInput")
    bvcd = nc.dram_tensor("bvec", [P, 4], F32, kind="ExternalInput")
    parts = nc.dram_tensor("partials", [P, CHUNKS * 3], F32,
                           kind="ExternalOutput")
    pred_v = pred.rearrange("n (p s) c -> n p s c", s=4)
    targ_v = targ.rearrange("n (p s) c -> n p s c", s=4)

    with tile.TileContext(nc) as tc:
        with tc.tile_pool(name="bufs", bufs=1) as pool, \
             tc.tile_pool(name="ps", bufs=1, space="PSUM") as pp:
            sh4 = [P, 2, 4, W]
            NS = 2  # streams

            def per_stream(nm, shape, dt):
                return [pool.tile(shape, dt, name=f"{nm}{i}")
                        for i in range(NS)]

            xa = per_stream("xa", sh4, BF16)
            xb = per_stream("xb", sh4, BF16)
            t = per_stream("t", sh4, BF16)      # contour scratch
            m = per_stream("m", sh4, BF16)
            Mh = per_stream("Mh", sh4, BF16)
            sk = per_stream("sk", sh4, BF16)
            t5 = per_stream("t5", [P, 2, 5, W], BF16)
            stage = per_stream("stage", sh4, F32)
            qu = per_stream("qu", [P, 2, W], BF16)
            qd = per_stream("qd", [P, 2, W], BF16)
            # shared hpool pair scratch: +/-BIG pad cols 0,512
            tmin = pool.tile([P, 2, 4, W + 1], BF16)
            tmax = pool.tile([P, 2, 4, W + 1], BF16)
            sup = pool.tile([P, P], BF16)
            sdn = pool.tile([P, P], BF16)
            bvec = pool.tile([P, 4], F32)
            pt = pool.tile([P, CHUNKS * 3], F32)
            pu = [pp.tile([P, 2, W], F32, name=f"pu{i}") for i in range(NS)]
            pd = [pp.tile([P, 2, W], F32, name=f"pd{i}") for i in range(NS)]

            nc.sync.dma_start(out=sup, in_=supd[:])
            nc.sync.dma_start(out=sdn, in_=sdnd[:])
            nc.sync.dma_start(out=bvec, in_=bvcd[:])
            nc.vector.memset(tmin[:, :, :, 0:1], BIG)
            nc.vector.memset(tmin[:, :, :, W:W + 1], BIG)
            nc.vector.memset(tmax[:, :, :, 0:1], -BIG)
            nc.vector.memset(tmax[:, :, :, W:W + 1], -BIG)

            def tt(out, a, b, op):
                nc.vector.tensor_tensor(out=out, in0=a, in1=b, op=op)

            def hpool(dst, src, op, by_side=False):
                # dst = 3-wide col pool of src (SAME, clipped). The pair
                # temp has static +/-BIG pad cols, so the second op covers
                # the edge columns too.  The combine is split so slots 3/0
                # land first: they feed the next vpool's shift matmuls.
                # by_side splits the pair op so side 0 (pred) can start
                # before side 1 (targ) finishes loading.
                tp = tmin if op == OP.min else tmax
                if by_side:
                    tt(tp[:, 0, :, 1:512], src[:, 0, :, 0:511],
                       src[:, 0, :, 1:512], op)
                    tt(tp[:, 1, :, 1:512], src[:, 1, :, 0:511],
                       src[:, 1, :, 1:512], op)
                else:
                    tt(tp[:, :, :, 1:512], src[:, :, :, 0:511],
                       src[:, :, :, 1:512], op)
                tt(dst[:, :, 3, 0:512], tp[:, :, 3, 0:512],
                   tp[:, :, 3, 1:513], op)
                tt(dst[:, :, 0, 0:512], tp[:, :, 0, 0:512],
                   tp[:, :, 0, 1:513], op)
                tt(dst[:, :, 1:3, 0:512], tp[:, :, 1:3, 0:512],
                   tp[:, :, 1:3, 1:513], op)

            def vpool(s, dst, src, op):
                # dst = 3-wide row pool of src across partitions;
                # t5 = [shift-up, pair01, pair12, pair23, shift-dn].
                bc = 0 if op == OP.min else 2
                t5s, pus, pds = t5[s], pu[s], pd[s]
                nc.tensor.matmul(pus[:, 0], sup[:], src[:, 0, 3, :])
                nc.tensor.matmul(pus[:, 1], sup[:], src[:, 1, 3, :])
                nc.scalar.activation(out=t5s[:, :, 0, :], in_=pus,
                                     func=AF.Identity,
                                     bias=bvec[:, bc:bc + 1])  # f32 -> bf16
                nc.tensor.matmul(pds[:, 0], sdn[:], src[:, 0, 0, :])
                nc.tensor.matmul(pds[:, 1], sdn[:], src[:, 1, 0, :])
                nc.scalar.activation(out=t5s[:, :, 4, :], in_=pds,
                                     func=AF.Identity,
                                     bias=bvec[:, bc + 1:bc + 2])
                tt(t5s[:, :, 1:4, :], src[:, :, 0:3, :],
                   src[:, :, 1:4, :], op)
                tt(dst[:, :, 0:4, :], t5s[:, :, 0:4, :],
                   t5s[:, :, 1:5, :], op)

            # stream state: (cur_x, other)
            state = [None, None]

            def emit_load(s, ch):
                # one DMA per row-slot: queue entries spread across the DMA
                # engines, so smaller transfers raise effective bandwidth
                st = stage[s]
                qs = [nc.sync, nc.scalar, nc.sync, nc.scalar]
                for sl in range(4):
                    qs[sl].dma_start(out=st[:, 0, sl:sl + 1],
                                     in_=pred_v[ch, :, sl:sl + 1])
                for sl in range(4):
                    nc.gpsimd.dma_start(out=st[:, 1, sl:sl + 1],
                                        in_=targ_v[ch, :, sl:sl + 1])
                x = xa[s]
                nc.scalar.copy(out=x[:, 0], in_=st[:, 0])  # f32 -> bf16
                nc.scalar.copy(out=x[:, 1], in_=st[:, 1])
                state[s] = (x, xb[s])

            def emit_iter(s, it):
                x, other = state[s]
                mh = other
                hpool(mh, x, OP.min, by_side=(it == 0))
                vpool(s, m[s], mh, OP.min)
                hpool(mh, m[s], OP.max)
                vpool(s, Mh[s], mh, OP.max)
                tt(t[s][:, :, :, :], Mh[s][:, :, :, :], m[s][:, :, :, :],
                   OP.subtract)          # contour
                out_x = sk[s] if it == ITERS - 1 else mh
                tt(out_x[:, :, :, :], x[:, :, :, :], t[s][:, :, :, :],
                   OP.subtract)
                if it < ITERS - 1:
                    state[s] = (mh, x)

            def emit_post(s, ch):
                # post tensors alias this stream's dead morphology buffers
                sks = sk[s]
                scr, shb, ncb, onb = m[s], Mh[s], state[s][1], t[s]
                # ncnt = 3x3 sum-pool of sk, all bf16
                tt(scr[:, :, :, 0:511], sks[:, :, :, 0:511],
                   sks[:, :, :, 1:512], OP.add)
                tt(shb[:, :, 3, 1:511], scr[:, :, 3, 0:510],
                   sks[:, :, 3, 2:512], OP.add)
                tt(shb[:, :, 0, 1:511], scr[:, :, 0, 0:510],
                   sks[:, :, 0, 2:512], OP.add)
                tt(shb[:, :, 1:3, 1:511], scr[:, :, 1:3, 0:510],
                   sks[:, :, 1:3, 2:512], OP.add)
                nc.scalar.copy(out=shb[:, :, :, 0:1], in_=scr[:, :, :, 0:1])
                nc.scalar.copy(out=shb[:, :, :, 511:512],
                               in_=scr[:, :, :, 510:511])
                # vertical sum via slot pairs + cross-partition shift rows
                nc.tensor.matmul(pu[s][:, 0], sup[:], shb[:, 0, 3, :])
                nc.tensor.matmul(pu[s][:, 1], sup[:], shb[:, 1, 3, :])
                nc.scalar.copy(out=qu[s], in_=pu[s])          # f32 -> bf16
                nc.tensor.matmul(pd[s][:, 0], sdn[:], shb[:, 0, 0, :])
                nc.tensor.matmul(pd[s][:, 1], sdn[:], shb[:, 1, 0, :])
                nc.scalar.copy(out=qd[s], in_=pd[s])  # row127 = 0 (clipped)
                tt(scr[:, :, 1:4, :], shb[:, :, 0:3, :], shb[:, :, 1:4, :],
                   OP.add)
                tt(ncb[:, :, 1:3, :], scr[:, :, 1:3, :], shb[:, :, 2:4, :],
                   OP.add)
                tt(ncb[:, :, 0, :], scr[:, :, 1, :], qu[s][:], OP.add)
                tt(ncb[:, :, 3, :], scr[:, :, 3, :], qd[s][:], OP.add)
                # on = sk > 0.5 ; ep = (ncnt == 2)*on ; cr = (ncnt >= 4)*on
                # (tensor_scalar runs 4x on bf16; masks multiply in place)
                nc.vector.tensor_scalar(out=onb[:, :, :, :],
                                        in0=sks[:, :, :, :],
                                        scalar1=0.5, scalar2=None,
                                        op0=OP.is_gt)
                # squared-diff partial sums: diff on DVE (bf16 2x),
                # square + row-sum on ScalarE (Square + accum_out, f32)
                tt(scr[:, 0], sks[:, 0], sks[:, 1], OP.subtract)
                nc.scalar.activation(
                    out=scr[:, 1], in_=scr[:, 0], func=AF.Square,
                    accum_out=pt[:, ch * 3: ch * 3 + 1])
                for k, op0 in ((1, OP.is_equal), (2, OP.is_ge)):
                    nc.vector.tensor_scalar(out=shb[:, :, :, :],
                                            in0=ncb[:, :, :, :],
                                            scalar1=2.0 if k == 1 else 4.0,
                                            scalar2=None, op0=op0)
                    tt(shb[:, :, :, :], shb[:, :, :, :], onb[:, :, :, :],
                       OP.mult)
                    tt(scr[:, 0], shb[:, 0], shb[:, 1], OP.subtract)
                    nc.scalar.activation(
                        out=scr[:, 1], in_=scr[:, 0], func=AF.Square,
                        accum_out=pt[:, ch * 3 + k: ch * 3 + k + 1])
                # stream this chunk's partials out
                nc.sync.dma_start(out=parts[:, ch * 3: ch * 3 + 3],
                                  in_=pt[:, ch * 3: ch * 3 + 3])

            for pair in range(CHUNKS // 2):
                chA, chB = 2 * pair, 2 * pair + 1
                emit_load(0, chA)
                emit_load(1, chB)
                for it in range(ITERS):
                    emit_iter(0, it)
                    emit_iter(1, it)
                emit_post(0, chA)
                emit_post(1, chB)

    _split_waits(nc, limit=1)
    return nc


def _run(pred_np, targ_np, trace=False):
    if "nc" not in _cache:
        _cache["nc"] = _build()
    nc = _cache["nc"]
    sup, sdn, bvec = _shift_mats()
    in_maps = []
    for c in range(NCORES):
        in_maps.append({
            "pred": np.ascontiguousarray(pred_np[c * CHUNKS:(c + 1) * CHUNKS]),
            "targ": np.ascontiguousarray(targ_np[c * CHUNKS:(c + 1) * CHUNKS]),
            "sup": sup, "sdn": sdn, "bvec": bvec,
        })
    return run_bass_kernel_spmd(nc, in_maps, core_ids=list(range(NCORES)),
                                trace=trace)


def kernel(pred, target):
    pred_np = np.asarray(pred, dtype=np.float32).reshape(32, H, W)
    targ_np = np.asarray(target, dtype=np.float32).reshape(32, H, W)
    res = _run(pred_np, targ_np)
    sums = np.zeros(3, dtype=np.float64)
    for r in res.results:
        p = r["partials"].astype(np.float64).reshape(P, CHUNKS, 3)
        sums += p.sum(axis=(0, 1))
    n = 32.0 * H * W
    loss = 0.6 * sums[0] / n + 0.2 * sums[1] / n + 0.2 * sums[2] / n
    return np.float32(loss)


# revision 30
# speedup vs baseline: 1.0182x; 1.0182x over previous
"""ConnectivityLoss Trainium2 Bass kernel.

Problem (hardcoded): pred/target (32, 1, 512, 512) f32.
  5 iterations of soft-skeletonize (3x3 min-pool -> 3x3 max-pool ->
  x = x - (M - m); both reference relus are provably no-ops), then 3x3
  sum-pool, endpoint/crossing masks, and a weighted MSE of the three
  pairs.

Sharding: pure data parallel over the batch dim; core i processes image
pairs 4i..4i+3 and returns per-partition partial sums of squared diffs;
the host sums and normalizes.

Per-core layout: partition p (128) owns image rows 4p..4p+3.
Free dims: (side 2, rowslot 4, col 512), fully contiguous.

Everything on-device runs in bf16 (numpy-validated loss error ~5.7e-3
vs the 2e-2 gate).  bf16 keeps every DVE tensor_tensor in the 2x perf
mode (f32 tt runs 1x).  MSE squares+row-sums run on the Scalar engine
(ACT Square with accum_out).

The four chunks are processed as two interleaved streams (chunk pairs
(0,1) and (2,3)): per iteration the emission alternates stream A /
stream B so each stream's TensorE shift-matmul + ACT evacuation
latencies hide under the other stream's DVE block, and each stream's
post-pool overlaps the other's morphology.  Each stream owns its
x/m/M/sk/t5/stage buffers; the hpool pair scratch (tmin/tmax) is
shared (adjacent-op WAR, zero cost).  Post-pool tensors alias the
stream's dead morphology buffers.

Cross-partition row shifts (rows 4p-1 / 4p+4) run on the idle
TensorEngine as shifted-identity bf16 matmuls into PSUM; ScalarE
evacuates PSUM to bf16 SBUF rows via an Identity activation whose
per-partition bias plants +/-BIG sentinels at the image boundary rows
(the shift matrices write exact 0.0 there), so pool combines cover all
128 partitions with no boundary special case.  The hpool combine is
split into slot3 / slot0 / slots1:2 ops so the following vpool's shift
matmuls start ~2us early.
"""
import numpy as np
import ml_dtypes

import concourse.bass as bass
import concourse.tile as tile
from concourse import mybir
from concourse.bass_utils import run_bass_kernel_spmd

F32 = mybir.dt.float32
BF16 = mybir.dt.bfloat16
OP = mybir.AluOpType
AF = mybir.ActivationFunctionType

BIG = 1.0e30
P = 128
NCORES = 8
CHUNKS = 4
H = W = 512
ITERS = 5

_cache = {}


def _split_waits(nc, limit=1):
    """This walrus build rejects instructions with more than ~1 embedded
    sync wait; hoist waits into standalone EventSemaphore instructions."""
    counter = 0
    for fn in nc.m.functions:
        for bb in fn.blocks:
            lst = list(bb.instructions)
            out = []
            changed = False
            for ins in lst:
                si = ins.sync_info
                waits = list(si.on_wait) if si is not None else []
                if len(waits) > limit:
                    changed = True
                    for w in waits:
                        counter += 1
                        es = mybir.InstEventSemaphore(
                            name=f"I-wsplit-{counter}", ins=[], outs=[],
                            sync_info=mybir.SyncInfo(on_wait=[w], on_update=[]),
                            bass_nofuse=True,
                        )
                        es.engine = ins.engine
                        out.append(es)
                    ins.sync_info = mybir.SyncInfo(
                        on_wait=[], on_update=list(si.on_update))
                out.append(ins)
            if changed:
                bb.instructions = out
    return counter


def _shift_mats():
    sup = np.zeros((P, P), np.float32)   # psum[p] = rhs[p-1]; col 0 zero
    sdn = np.zeros((P, P), np.float32)   # psum[p] = rhs[p+1]; col 127 zero
    for p in range(1, P):
        sup[p - 1, p] = 1.0
    for p in range(P - 1):
        sdn[p + 1, p] = 1.0
    bvec = np.zeros((P, 4), np.float32)
    bvec[0, 0] = BIG      # min, shift-up sentinel at row 0
    bvec[127, 1] = BIG    # min, shift-down sentinel at row 511
    bvec[0, 2] = -BIG     # max
    bvec[127, 3] = -BIG
    return (sup.astype(ml_dtypes.bfloat16), sdn.astype(ml_dtypes.bfloat16),
            bvec)


def _build():
    nc = bass.Bass()
    # inputs arrive pre-rounded to bf16 (host astype rounds identically to
    # the ACT f32->bf16 copy this replaces): half the DMA bytes, no staging
    pred = nc.dram_tensor("pred", [CHUNKS, H, W], BF16, kind="ExternalInput")
    targ = nc.dram_tensor("targ", [CHUNKS, H, W], BF16, kind="ExternalInput")
    supd = nc.dram_tensor("sup", [P, P], BF16, kind="ExternalInput")
    sdnd = nc.dram_tensor("sdn", [P, P], BF16, kind="ExternalInput")
    bvcd = nc.dram_tensor("bvec", [P, 4], F32, kind="ExternalInput")
    parts = nc.dram_tensor("partials", [P, CHUNKS * 3], F32,
                           kind="ExternalOutput")
    pred_v = pred.rearrange("n (p s) c -> n p s c", s=4)
    targ_v = targ.rearrange("n (p s) c -> n p s c", s=4)

    with tile.TileContext(nc) as tc:
        with tc.tile_pool(name="bufs", bufs=1) as pool, \
             tc.tile_pool(name="ps", bufs=1, space="PSUM") as pp:
            sh4 = [P, 2, 4, W]
            NS = 2  # streams

            def per_stream(nm, shape, dt):
                return [pool.tile(shape, dt, name=f"{nm}{i}")
                        for i in range(NS)]

            xa = per_stream("xa", sh4, BF16)
            xb = per_stream("xb", sh4, BF16)
            t = per_stream("t", sh4, BF16)      # contour scratch
            m = per_stream("m", sh4, BF16)
            Mh = per_stream("Mh", sh4, BF16)
            sk = per_stream("sk", sh4, BF16)
            t5 = per_stream("t5", [P, 2, 5, W], BF16)
            qu = per_stream("qu", [P, 2, W], BF16)
            qd = per_stream("qd", [P, 2, W], BF16)
            # shared hpool pair scratch: +/-BIG pad cols 0,512
            tmin = pool.tile([P, 2, 4, W + 1], BF16)
            tmax = pool.tile([P, 2, 4, W + 1], BF16)
            sup = pool.tile([P, P], BF16)
            sdn = pool.tile([P, P], BF16)
            bvec = pool.tile([P, 4], F32)
            pt = pool.tile([P, CHUNKS * 3], F32)
            pu = [pp.tile([P, 2, W], F32, name=f"pu{i}") for i in range(NS)]
            pd = [pp.tile([P, 2, W], F32, name=f"pd{i}") for i in range(NS)]

            nc.sync.dma_start(out=sup, in_=supd[:])
            nc.sync.dma_start(out=sdn, in_=sdnd[:])
            nc.sync.dma_start(out=bvec, in_=bvcd[:])
            nc.vector.memset(tmin[:, :, :, 0:1], BIG)
            nc.vector.memset(tmin[:, :, :, W:W + 1], BIG)
            nc.vector.memset(tmax[:, :, :, 0:1], -BIG)
            nc.vector.memset(tmax[:, :, :, W:W + 1], -BIG)

            def tt(out, a, b, op):
                nc.vector.tensor_tensor(out=out, in0=a, in1=b, op=op)

            def hpool(dst, src, op, by_side=False):
                # dst = 3-wide col pool of src (SAME, clipped). The pair
                # temp has static +/-BIG pad cols, so the second op covers
                # the edge columns too.  The combine is split so slots 3/0
                # land first: they feed the next vpool's shift matmuls.
                # by_side splits the pair op so side 0 (pred) can start
                # before side 1 (targ) finishes loading.
                tp = tmin if op == OP.min else tmax
                if by_side:
                    tt(tp[:, 0, :, 1:512], src[:, 0, :, 0:511],
                       src[:, 0, :, 1:512], op)
                    tt(tp[:, 1, :, 1:512], src[:, 1, :, 0:511],
                       src[:, 1, :, 1:512], op)
                else:
                    tt(tp[:, :, :, 1:512], src[:, :, :, 0:511],
                       src[:, :, :, 1:512], op)
                tt(dst[:, :, 3, 0:512], tp[:, :, 3, 0:512],
                   tp[:, :, 3, 1:513], op)
                tt(dst[:, :, 0, 0:512], tp[:, :, 0, 0:512],
                   tp[:, :, 0, 1:513], op)
                tt(dst[:, :, 1:3, 0:512], tp[:, :, 1:3, 0:512],
                   tp[:, :, 1:3, 1:513], op)

            def vpool(s, dst, src, op):
                # dst = 3-wide row pool of src across partitions;
                # t5 = [shift-up, pair01, pair12, pair23, shift-dn].
                bc = 0 if op == OP.min else 2
                t5s, pus, pds = t5[s], pu[s], pd[s]
                nc.tensor.matmul(pus[:, 0], sup[:], src[:, 0, 3, :])
                nc.tensor.matmul(pus[:, 1], sup[:], src[:, 1, 3, :])
                nc.scalar.activation(out=t5s[:, :, 0, :], in_=pus,
                                     func=AF.Identity,
                                     bias=bvec[:, bc:bc + 1])  # f32 -> bf16
                nc.tensor.matmul(pds[:, 0], sdn[:], src[:, 0, 0, :])
                nc.tensor.matmul(pds[:, 1], sdn[:], src[:, 1, 0, :])
                nc.scalar.activation(out=t5s[:, :, 4, :], in_=pds,
                                     func=AF.Identity,
                                     bias=bvec[:, bc + 1:bc + 2])
                tt(t5s[:, :, 1:4, :], src[:, :, 0:3, :],
                   src[:, :, 1:4, :], op)
                tt(dst[:, :, 0:4, :], t5s[:, :, 0:4, :],
                   t5s[:, :, 1:5, :], op)

            # stream state: (cur_x, other)
            state = [None, None]

            def emit_load(s, ch):
                # bf16 DMAs land directly in the stream's x buffer
                x = xa[s]
                nc.sync.dma_start(out=x[:, 0, 0:2], in_=pred_v[ch, :, 0:2])
                nc.scalar.dma_start(out=x[:, 0, 2:4],
                                    in_=pred_v[ch, :, 2:4])
                nc.gpsimd.dma_start(out=x[:, 1, 0:2],
                                    in_=targ_v[ch, :, 0:2])
                nc.sync.dma_start(out=x[:, 1, 2:4],
                                  in_=targ_v[ch, :, 2:4])
                state[s] = (x, xb[s])

            def emit_iter(s, it):
                x, other = state[s]
                mh = other
                hpool(mh, x, OP.min, by_side=(it == 0))
                vpool(s, m[s], mh, OP.min)
                hpool(mh, m[s], OP.max)
                vpool(s, Mh[s], mh, OP.max)
                tt(t[s][:, :, :, :], Mh[s][:, :, :, :], m[s][:, :, :, :],
                   OP.subtract)          # contour
                out_x = sk[s] if it == ITERS - 1 else mh
                tt(out_x[:, :, :, :], x[:, :, :, :], t[s][:, :, :, :],
                   OP.subtract)
                if it < ITERS - 1:
                    state[s] = (mh, x)

            def emit_post(s, ch):
                # post tensors alias this stream's dead morphology buffers
                sks = sk[s]
                scr, shb, ncb, onb = m[s], Mh[s], state[s][1], t[s]
                # ncnt = 3x3 sum-pool of sk, all bf16
                tt(scr[:, :, :, 0:511], sks[:, :, :, 0:511],
                   sks[:, :, :, 1:512], OP.add)
                tt(shb[:, :, 3, 1:511], scr[:, :, 3, 0:510],
                   sks[:, :, 3, 2:512], OP.add)
                tt(shb[:, :, 0, 1:511], scr[:, :, 0, 0:510],
                   sks[:, :, 0, 2:512], OP.add)
                tt(shb[:, :, 1:3, 1:511], scr[:, :, 1:3, 0:510],
                   sks[:, :, 1:3, 2:512], OP.add)
                nc.scalar.copy(out=shb[:, :, :, 0:1], in_=scr[:, :, :, 0:1])
                nc.scalar.copy(out=shb[:, :, :, 511:512],
                               in_=scr[:, :, :, 510:511])
                # vertical sum via slot pairs + cross-partition shift rows
                nc.tensor.matmul(pu[s][:, 0], sup[:], shb[:, 0, 3, :])
                nc.tensor.matmul(pu[s][:, 1], sup[:], shb[:, 1, 3, :])
                nc.scalar.copy(out=qu[s], in_=pu[s])          # f32 -> bf16
                nc.tensor.matmul(pd[s][:, 0], sdn[:], shb[:, 0, 0, :])
                nc.tensor.matmul(pd[s][:, 1], sdn[:], shb[:, 1, 0, :])
                nc.scalar.copy(out=qd[s], in_=pd[s])  # row127 = 0 (clipped)
                tt(scr[:, :, 1:4, :], shb[:, :, 0:3, :], shb[:, :, 1:4, :],
                   OP.add)
                tt(ncb[:, :, 1:3, :], scr[:, :, 1:3, :], shb[:, :, 2:4, :],
                   OP.add)
                tt(ncb[:, :, 0, :], scr[:, :, 1, :], qu[s][:], OP.add)
                tt(ncb[:, :, 3, :], scr[:, :, 3, :], qd[s][:], OP.add)
                # on = sk > 0.5 ; ep = (ncnt == 2)*on ; cr = (ncnt >= 4)*on
                # (tensor_scalar runs 4x on bf16; masks multiply in place)
                nc.vector.tensor_scalar(out=onb[:, :, :, :],
                                        in0=sks[:, :, :, :],
                                        scalar1=0.5, scalar2=None,
                                        op0=OP.is_gt)
                # squared-diff partial sums: diff on DVE (bf16 2x),
                # square + row-sum on ScalarE (Square + accum_out, f32)
                tt(scr[:, 0], sks[:, 0], sks[:, 1], OP.subtract)
                nc.scalar.activation(
                    out=scr[:, 1], in_=scr[:, 0], func=AF.Square,
                    accum_out=pt[:, ch * 3: ch * 3 + 1])
                for k, op0 in ((1, OP.is_equal), (2, OP.is_ge)):
                    nc.vector.tensor_scalar(out=shb[:, :, :, :],
                                            in0=ncb[:, :, :, :],
                                            scalar1=2.0 if k == 1 else 4.0,
                                            scalar2=None, op0=op0)
                    tt(shb[:, :, :, :], shb[:, :, :, :], onb[:, :, :, :],
                       OP.mult)
                    tt(scr[:, 0], shb[:, 0], shb[:, 1], OP.subtract)
                    nc.scalar.activation(
                        out=scr[:, 1], in_=scr[:, 0], func=AF.Square,
                        accum_out=pt[:, ch * 3 + k: ch * 3 + k + 1])
                # stream this chunk's partials out
                nc.sync.dma_start(out=parts[:, ch * 3: ch * 3 + 3],
                                  in_=pt[:, ch * 3: ch * 3 + 3])

            for pair in range(CHUNKS // 2):
                chA, chB = 2 * pair, 2 * pair + 1
                emit_load(0, chA)
                emit_load(1, chB)
                for it in range(ITERS):
                    emit_iter(0, it)
                    emit_iter(1, it)
                emit_post(0, chA)
                emit_post(1, chB)

    _split_waits(nc, limit=1)
    return nc


def _run(pred_np, targ_np, trace=False):
    if "nc" not in _cache:
        _cache["nc"] = _build()
    nc = _cache["nc"]
    sup, sdn, bvec = _shift_mats()
    in_maps = []
    for c in range(NCORES):
        in_maps.append({
            "pred": np.ascontiguousarray(pred_np[c * CHUNKS:(c + 1) * CHUNKS]),
            "targ": np.ascontiguousarray(targ_np[c * CHUNKS:(c + 1) * CHUNKS]),
            "sup": sup, "sdn": sdn, "bvec": bvec,
        })
    return run_bass_kernel_spmd(nc, in_maps, core_ids=list(range(NCORES)),
                                trace=trace)


def kernel(pred, target):
    pred_np = np.asarray(pred, dtype=np.float32).reshape(32, H, W) \
        .astype(ml_dtypes.bfloat16)
    targ_np = np.asarray(target, dtype=np.float32).reshape(32, H, W) \
        .astype(ml_dtypes.bfloat16)
    res = _run(pred_np, targ_np)
    sums = np.zeros(3, dtype=np.float64)
    for r in res.results:
        p = r["partials"].astype(np.float64).reshape(P, CHUNKS, 3)
        sums += p.sum(axis=(0, 1))
    n = 32.0 * H * W
    loss = 0.6 * sums[0] / n + 0.2 * sums[1] / n + 0.2 * sums[2] / n
    return np.float32(loss)


# revision 38
# speedup vs baseline: 1.0892x; 1.0698x over previous
"""ConnectivityLoss Trainium2 Bass kernel.

Problem (hardcoded): pred/target (32, 1, 512, 512) f32.
  5 iterations of soft-skeletonize (3x3 min-pool -> 3x3 max-pool ->
  x = x - (M - m); both reference relus are provably no-ops), then 3x3
  sum-pool, endpoint/crossing masks, and a weighted MSE of the three
  pairs.

Sharding: pure data parallel over the batch dim; core i processes image
pairs 4i..4i+3 and returns per-partition partial sums of squared diffs;
the host sums and normalizes.

Per-core layout: partition p (128) owns image rows 4p..4p+3.
Free dims: (side 2, rowslot 4, col 512), fully contiguous.

Everything on-device runs in bf16 (numpy-validated loss error ~5.7e-3
vs the 2e-2 gate).  bf16 keeps every DVE tensor_tensor in the 2x perf
mode (f32 tt runs 1x).  MSE squares+row-sums run on the Scalar engine
(ACT Square with accum_out).

The four chunks are processed as two interleaved streams (chunk pairs
(0,1) and (2,3)): per iteration the emission alternates stream A /
stream B so each stream's TensorE shift-matmul + ACT evacuation
latencies hide under the other stream's DVE block, and each stream's
post-pool overlaps the other's morphology.  Each stream owns its
x/m/M/sk/t5/stage buffers; the hpool pair scratch (tmin/tmax) is
shared (adjacent-op WAR, zero cost).  Post-pool tensors alias the
stream's dead morphology buffers.

Cross-partition row shifts (rows 4p-1 / 4p+4) run on the idle
TensorEngine as shifted-identity bf16 matmuls into PSUM; ScalarE
evacuates PSUM to bf16 SBUF rows via an Identity activation whose
per-partition bias plants +/-BIG sentinels at the image boundary rows
(the shift matrices write exact 0.0 there), so pool combines cover all
128 partitions with no boundary special case.  The hpool combine is
split into slot3 / slot0 / slots1:2 ops so the following vpool's shift
matmuls start ~2us early.
"""
import numpy as np
import ml_dtypes

import concourse.bass as bass
import concourse.tile as tile
from concourse import mybir
from concourse.bass_utils import run_bass_kernel_spmd

F32 = mybir.dt.float32
BF16 = mybir.dt.bfloat16
OP = mybir.AluOpType
AF = mybir.ActivationFunctionType

BIG = 1.0e30
P = 128
NCORES = 8
CHUNKS = 4
H = W = 512
ITERS = 5

_cache = {}


def _split_waits(nc, limit=1):
    """This walrus build rejects instructions with more than ~1 embedded
    sync wait; hoist waits into standalone EventSemaphore instructions."""
    counter = 0
    for fn in nc.m.functions:
        for bb in fn.blocks:
            lst = list(bb.instructions)
            out = []
            changed = False
            for ins in lst:
                si = ins.sync_info
                waits = list(si.on_wait) if si is not None else []
                if len(waits) > limit:
                    changed = True
                    for w in waits:
                        counter += 1
                        es = mybir.InstEventSemaphore(
                            name=f"I-wsplit-{counter}", ins=[], outs=[],
                            sync_info=mybir.SyncInfo(on_wait=[w], on_update=[]),
                            bass_nofuse=True,
                        )
                        es.engine = ins.engine
                        out.append(es)
                    ins.sync_info = mybir.SyncInfo(
                        on_wait=[], on_update=list(si.on_update))
                out.append(ins)
            if changed:
                bb.instructions = out
    return counter


def _shift_mats():
    sup = np.zeros((P, P), np.float32)   # psum[p] = rhs[p-1]; col 0 zero
    sdn = np.zeros((P, P), np.float32)   # psum[p] = rhs[p+1]; col 127 zero
    for p in range(1, P):
        sup[p - 1, p] = 1.0
    for p in range(P - 1):
        sdn[p + 1, p] = 1.0
    bvec = np.zeros((P, 4), np.float32)
    bvec[0, 0] = BIG      # min, shift-up sentinel at row 0
    bvec[127, 1] = BIG    # min, shift-down sentinel at row 511
    bvec[0, 2] = -BIG     # max
    bvec[127, 3] = -BIG
    return (sup.astype(ml_dtypes.bfloat16), sdn.astype(ml_dtypes.bfloat16),
            np.eye(P, dtype=np.float32).astype(ml_dtypes.bfloat16), bvec)


def _build():
    nc = bass.Bass()
    # inputs arrive pre-rounded to bf16 (host astype rounds identically to
    # the ACT f32->bf16 copy this replaces): half the DMA bytes, no staging
    pred = nc.dram_tensor("pred", [CHUNKS, H, W], BF16, kind="ExternalInput")
    targ = nc.dram_tensor("targ", [CHUNKS, H, W], BF16, kind="ExternalInput")
    supd = nc.dram_tensor("sup", [P, P], BF16, kind="ExternalInput")
    sdnd = nc.dram_tensor("sdn", [P, P], BF16, kind="ExternalInput")
    idnd = nc.dram_tensor("idn", [P, P], BF16, kind="ExternalInput")
    bvcd = nc.dram_tensor("bvec", [P, 4], F32, kind="ExternalInput")
    parts = nc.dram_tensor("partials", [P, CHUNKS * 3], F32,
                           kind="ExternalOutput")
    pred_v = pred.rearrange("n (p s) c -> n p s c", s=4)
    targ_v = targ.rearrange("n (p s) c -> n p s c", s=4)

    with tile.TileContext(nc) as tc:
        with tc.tile_pool(name="bufs", bufs=1) as pool, \
             tc.tile_pool(name="ps", bufs=1, space="PSUM") as pp:
            sh4 = [P, 2, 4, W]
            NS = 2  # streams

            def per_stream(nm, shape, dt):
                return [pool.tile(shape, dt, name=f"{nm}{i}")
                        for i in range(NS)]

            xa = per_stream("xa", sh4, BF16)
            xb = per_stream("xb", sh4, BF16)
            t = per_stream("t", sh4, BF16)      # contour scratch
            m = per_stream("m", sh4, BF16)
            Mh = per_stream("Mh", sh4, BF16)
            sk = per_stream("sk", sh4, BF16)
            t5 = per_stream("t5", [P, 2, 5, W], BF16)
            qu = per_stream("qu", [P, 2, W], BF16)
            qd = per_stream("qd", [P, 2, W], BF16)
            # shared hpool pair scratch: +/-BIG pad cols 0,512
            tmin = pool.tile([P, 2, 4, W + 1], BF16)
            tmax = pool.tile([P, 2, 4, W + 1], BF16)
            sup = pool.tile([P, P], BF16)
            sdn = pool.tile([P, P], BF16)
            idn = pool.tile([P, P], BF16)
            bvec = pool.tile([P, 4], F32)
            pt = pool.tile([P, CHUNKS * 3], F32)
            pu = [pp.tile([P, 2, W], F32, name=f"pu{i}") for i in range(NS)]
            pd = [pp.tile([P, 2, W], F32, name=f"pd{i}") for i in range(NS)]

            nc.sync.dma_start(out=sup, in_=supd[:])
            nc.sync.dma_start(out=sdn, in_=sdnd[:])
            nc.sync.dma_start(out=idn, in_=idnd[:])
            nc.sync.dma_start(out=bvec, in_=bvcd[:])
            nc.vector.memset(tmin[:, :, :, 0:1], BIG)
            nc.vector.memset(tmin[:, :, :, W:W + 1], BIG)
            nc.vector.memset(tmax[:, :, :, 0:1], -BIG)
            nc.vector.memset(tmax[:, :, :, W:W + 1], -BIG)

            def tt(out, a, b, op):
                nc.vector.tensor_tensor(out=out, in0=a, in1=b, op=op)

            def hpool(dst, src, op, by_side=False):
                # dst = 3-wide col pool of src (SAME, clipped). The pair
                # temp has static +/-BIG pad cols, so the second op covers
                # the edge columns too.  The combine is split so slots 3/0
                # land first: they feed the next vpool's shift matmuls.
                # by_side splits the pair op so side 0 (pred) can start
                # before side 1 (targ) finishes loading.
                tp = tmin if op == OP.min else tmax
                if by_side:
                    tt(tp[:, 0, :, 1:512], src[:, 0, :, 0:511],
                       src[:, 0, :, 1:512], op)
                    tt(tp[:, 1, :, 1:512], src[:, 1, :, 0:511],
                       src[:, 1, :, 1:512], op)
                else:
                    tt(tp[:, :, :, 1:512], src[:, :, :, 0:511],
                       src[:, :, :, 1:512], op)
                tt(dst[:, :, 3, 0:512], tp[:, :, 3, 0:512],
                   tp[:, :, 3, 1:513], op)
                tt(dst[:, :, 0, 0:512], tp[:, :, 0, 0:512],
                   tp[:, :, 0, 1:513], op)
                tt(dst[:, :, 1:3, 0:512], tp[:, :, 1:3, 0:512],
                   tp[:, :, 1:3, 1:513], op)

            def vpool(s, dst, src, op):
                # dst = 3-wide row pool of src across partitions;
                # t5 = [shift-up, pair01, pair12, pair23, shift-dn].
                bc = 0 if op == OP.min else 2
                t5s, pus, pds = t5[s], pu[s], pd[s]
                nc.tensor.matmul(pus[:, 0], sup[:], src[:, 0, 3, :])
                nc.tensor.matmul(pus[:, 1], sup[:], src[:, 1, 3, :])
                nc.scalar.activation(out=t5s[:, :, 0, :], in_=pus,
                                     func=AF.Identity,
                                     bias=bvec[:, bc:bc + 1])  # f32 -> bf16
                nc.tensor.matmul(pds[:, 0], sdn[:], src[:, 0, 0, :])
                nc.tensor.matmul(pds[:, 1], sdn[:], src[:, 1, 0, :])
                nc.scalar.activation(out=t5s[:, :, 4, :], in_=pds,
                                     func=AF.Identity,
                                     bias=bvec[:, bc + 1:bc + 2])
                tt(t5s[:, :, 1:4, :], src[:, :, 0:3, :],
                   src[:, :, 1:4, :], op)
                tt(dst[:, :, 0:4, :], t5s[:, :, 0:4, :],
                   t5s[:, :, 1:5, :], op)

            # stream state: (cur_x, other)
            state = [None, None]
            # deferred emissions: per-stream side-1 update matmuls and the
            # final subtract of the pending iteration (emitted later so the
            # in-order PE/DVE queues of the other stream fill the latency)
            pend_upd = [None, None]
            pend_sub = [None, None]

            def emit_upd_side(s, x, side):
                # pu/pd banks (free after this iter's vshift evacs) get
                # c = x + m via identity-matmul accumulation; ACT evacuates
                # to bf16 in t[s].  One DVE subtract then forms
                # x' = c - M  (= x - (M - m)).
                for sl in range(4):
                    dst = pu[s][:, sl, :] if sl < 2 else pd[s][:, sl - 2, :]
                    nc.tensor.matmul(dst, idn[:], x[:, side, sl, :],
                                     start=True, stop=False)
                    nc.tensor.matmul(dst, idn[:], m[s][:, side, sl, :],
                                     start=False, stop=True)
                nc.scalar.copy(out=t[s][:, side, 0:2], in_=pu[s])
                nc.scalar.copy(out=t[s][:, side, 2:4], in_=pd[s])

            def flush_upd(s):
                if pend_upd[s] is not None:
                    x, = pend_upd[s]
                    pend_upd[s] = None
                    emit_upd_side(s, x, 1)

            def flush_sub(s):
                if pend_sub[s] is not None:
                    out_x, = pend_sub[s]
                    pend_sub[s] = None
                    tt(out_x[:, :, :, :], t[s][:, :, :, :],
                       Mh[s][:, :, :, :], OP.subtract)

            def emit_load(s, ch):
                # bf16 DMAs land directly in the stream's x buffer
                x = xa[s]
                nc.sync.dma_start(out=x[:, 0, 0:2], in_=pred_v[ch, :, 0:2])
                nc.scalar.dma_start(out=x[:, 0, 2:4],
                                    in_=pred_v[ch, :, 2:4])
                nc.gpsimd.dma_start(out=x[:, 1, 0:2],
                                    in_=targ_v[ch, :, 0:2])
                nc.sync.dma_start(out=x[:, 1, 2:4],
                                  in_=targ_v[ch, :, 2:4])
                state[s] = (x, xb[s])

            def emit_iter(s, it):
                o = 1 - s
                flush_sub(s)             # previous iteration's x' lands
                x, other = state[s]
                mh = other
                hpool(mh, x, OP.min, by_side=(it == 0))
                vpool(s, m[s], mh, OP.min)
                flush_upd(o)             # other stream's side-1 update
                hpool(mh, m[s], OP.max)
                vpool(s, Mh[s], mh, OP.max)
                emit_upd_side(s, x, 0)
                pend_upd[s] = (x,)
                out_x = sk[s] if it == ITERS - 1 else mh
                pend_sub[s] = (out_x,)
                if it < ITERS - 1:
                    state[s] = (mh, x)

            def emit_post(s, ch):
                # drain this stream's deferred update/subtract (and the
                # other stream's side-1 update, so PE work is queued before
                # our DVE block) -- sk[s] is written by the final subtract
                flush_upd(1 - s)
                flush_upd(s)
                flush_sub(s)
                # post tensors alias this stream's dead morphology buffers
                sks = sk[s]
                scr, shb, ncb, onb = m[s], Mh[s], state[s][1], t[s]
                # ncnt = 3x3 sum-pool of sk, all bf16
                tt(scr[:, :, :, 0:511], sks[:, :, :, 0:511],
                   sks[:, :, :, 1:512], OP.add)
                tt(shb[:, :, 3, 1:511], scr[:, :, 3, 0:510],
                   sks[:, :, 3, 2:512], OP.add)
                tt(shb[:, :, 0, 1:511], scr[:, :, 0, 0:510],
                   sks[:, :, 0, 2:512], OP.add)
                tt(shb[:, :, 1:3, 1:511], scr[:, :, 1:3, 0:510],
                   sks[:, :, 1:3, 2:512], OP.add)
                nc.scalar.copy(out=shb[:, :, :, 0:1], in_=scr[:, :, :, 0:1])
                nc.scalar.copy(out=shb[:, :, :, 511:512],
                               in_=scr[:, :, :, 510:511])
                # vertical sum via slot pairs + cross-partition shift rows
                nc.tensor.matmul(pu[s][:, 0], sup[:], shb[:, 0, 3, :])
                nc.tensor.matmul(pu[s][:, 1], sup[:], shb[:, 1, 3, :])
                nc.scalar.copy(out=qu[s], in_=pu[s])          # f32 -> bf16
                nc.tensor.matmul(pd[s][:, 0], sdn[:], shb[:, 0, 0, :])
                nc.tensor.matmul(pd[s][:, 1], sdn[:], shb[:, 1, 0, :])
                nc.scalar.copy(out=qd[s], in_=pd[s])  # row127 = 0 (clipped)
                tt(scr[:, :, 1:4, :], shb[:, :, 0:3, :], shb[:, :, 1:4, :],
                   OP.add)
                tt(ncb[:, :, 1:3, :], scr[:, :, 1:3, :], shb[:, :, 2:4, :],
                   OP.add)
                tt(ncb[:, :, 0, :], scr[:, :, 1, :], qu[s][:], OP.add)
                tt(ncb[:, :, 3, :], scr[:, :, 3, :], qd[s][:], OP.add)
                # on = sk > 0.5 ; ep = (ncnt == 2)*on ; cr = (ncnt >= 4)*on
                # (tensor_scalar runs 4x on bf16; masks multiply in place)
                nc.vector.tensor_scalar(out=onb[:, :, :, :],
                                        in0=sks[:, :, :, :],
                                        scalar1=0.5, scalar2=None,
                                        op0=OP.is_gt)
                # squared-diff partial sums: diff on DVE (bf16 2x),
                # square + row-sum on ScalarE (Square + accum_out, f32)
                tt(scr[:, 0], sks[:, 0], sks[:, 1], OP.subtract)
                nc.scalar.activation(
                    out=scr[:, 1], in_=scr[:, 0], func=AF.Square,
                    accum_out=pt[:, ch * 3: ch * 3 + 1])
                for k, op0 in ((1, OP.is_equal), (2, OP.is_ge)):
                    nc.vector.tensor_scalar(out=shb[:, :, :, :],
                                            in0=ncb[:, :, :, :],
                                            scalar1=2.0 if k == 1 else 4.0,
                                            scalar2=None, op0=op0)
                    tt(shb[:, :, :, :], shb[:, :, :, :], onb[:, :, :, :],
                       OP.mult)
                    tt(scr[:, 0], shb[:, 0], shb[:, 1], OP.subtract)
                    nc.scalar.activation(
                        out=scr[:, 1], in_=scr[:, 0], func=AF.Square,
                        accum_out=pt[:, ch * 3 + k: ch * 3 + k + 1])
                # stream this chunk's partials out
                nc.sync.dma_start(out=parts[:, ch * 3: ch * 3 + 3],
                                  in_=pt[:, ch * 3: ch * 3 + 3])

            for pair in range(CHUNKS // 2):
                chA, chB = 2 * pair, 2 * pair + 1
                emit_load(0, chA)
                emit_load(1, chB)
                for it in range(ITERS):
                    emit_iter(0, it)
                    emit_iter(1, it)
                emit_post(0, chA)
                emit_post(1, chB)

    _split_waits(nc, limit=1)
    return nc


def _run(pred_np, targ_np, trace=False):
    if "nc" not in _cache:
        _cache["nc"] = _build()
    nc = _cache["nc"]
    sup, sdn, idn, bvec = _shift_mats()
    in_maps = []
    for c in range(NCORES):
        in_maps.append({
            "pred": np.ascontiguousarray(pred_np[c * CHUNKS:(c + 1) * CHUNKS]),
            "targ": np.ascontiguousarray(targ_np[c * CHUNKS:(c + 1) * CHUNKS]),
            "sup": sup, "sdn": sdn, "idn": idn, "bvec": bvec,
        })
    return run_bass_kernel_spmd(nc, in_maps, core_ids=list(range(NCORES)),
                                trace=trace)


def kernel(pred, target):
    pred_np = np.asarray(pred, dtype=np.float32).reshape(32, H, W) \
        .astype(ml_dtypes.bfloat16)
    targ_np = np.asarray(target, dtype=np.float32).reshape(32, H, W) \
        .astype(ml_dtypes.bfloat16)
    res = _run(pred_np, targ_np)
    sums = np.zeros(3, dtype=np.float64)
    for r in res.results:
        p = r["partials"].astype(np.float64).reshape(P, CHUNKS, 3)
        sums += p.sum(axis=(0, 1))
    n = 32.0 * H * W
    loss = 0.6 * sums[0] / n + 0.2 * sums[1] / n + 0.2 * sums[2] / n
    return np.float32(loss)
